# revision 22
# baseline (speedup 1.0000x reference)
"""Bidirectional complex-diagonal LRU (Linear Recurrent Unit) on 8 Trainium2 cores.

Math: lam = exp(-exp(nu_log) + i*exp(theta_log)) per channel n (N=512).
  Bu = einsum('blh,hn->bnl', u, B0 + iB1), masked to length.
  Forward scan over channels [0,256), backward (time-reversed) over [256,512).
  y = x.real @ C0 - x.imag @ C1, zeroed past each sequence length.

Device strategy (data-parallel, one batch per core):
  - Rotation trick: x_t = e^{i*th*t} * w_t turns the complex recurrence
    x_t = lam x_{t-1} + Bu_t into TWO real recurrences w_t = r w_{t-1} + v_t
    (r = |lam|), each a native DVE tensor_tensor_scan along the free dim.
  - Twiddle tables cos/sin(th*j) built on host in fp64 (exact phases), fp16 on
    device. Per-core masking (zero columns past the sequence length) is folded
    into the tables, so masking costs nothing on device.
  - Backward channels run on the reversed time axis; reversal happens inside
    the PSUM-evacuation copy (negative-stride AP) and the untwiddle writes.
  - All matmuls fp16 (full PE rate), accumulation in fp32 PSUM. Scans run
    in place over the v buffer; x overwrites v (fwd) / spent cos+sin table
    slices (bwd), so SBUF holds everything with no extra big buffers.
  - Elementwise combine ops are split DVE/GPSIMD to balance engine load.

Self-contained: hardcodes B=8, L=4096, H=N=512, 8 cores.
"""

import numpy as np
from contextlib import ExitStack

import concourse.bass as bass
import concourse.bacc as bacc
import concourse.mybir as mybir
import concourse.tile as tile

P = 128
L = 4096
H = 512
N = 512
BSZ = 8
SEG = 512                # Bu matmul / evac granularity (one PSUM bank)
NSEG = L // SEG          # 8
SLAB = 1024              # scan + untwiddle granularity
NSLAB = L // SLAB        # 4
TSLAB = 2048             # twiddle-in granularity
NTSLAB = L // TSLAB      # 2
KH = H // P              # 4 contraction chunks for Bu
NCH = 2 * N // P         # 8 real-channel chunks (re 0..3, im 4..7)
CCH = N // P             # 4 complex-channel chunks (0,1 fwd; 2,3 bwd)
NT = L // P              # 32 time blocks for the output matmul

F16 = mybir.dt.float16
F32 = mybir.dt.float32
MULT = mybir.AluOpType.mult
ADD = mybir.AluOpType.add

C_ORDER = [0, 2, 1, 3]

_CACHED = None


def _is_fwd_chunk(nch: int) -> bool:
    return (nch % 4) < 2


def build_nc():
    nc = bacc.Bacc("TRN2", target_bir_lowering=False, debug=False)
    uT = nc.declare_dram_parameter("uT", [H, L], F16, isOutput=False)
    cosT = nc.declare_dram_parameter("cosT", [N, L], F16, isOutput=False)
    sinT = nc.declare_dram_parameter("sinT", [N, L], F16, isOutput=False)
    rdec = nc.declare_dram_parameter("rdec", [P, CCH], F32, isOutput=False)
    Bcat = nc.declare_dram_parameter("Bcat", [H, 2 * N], F16, isOutput=False)
    Ccat = nc.declare_dram_parameter("Ccat", [2 * N, H], F16, isOutput=False)
    y = nc.declare_dram_parameter("y", [L, H], F32, isOutput=True)

    with tile.TileContext(nc) as tc, ExitStack() as ctx:
        const = ctx.enter_context(tc.tile_pool(name="const", bufs=1))
        big = ctx.enter_context(tc.tile_pool(name="big", bufs=1))
        upool = ctx.enter_context(tc.tile_pool(name="upool", bufs=1))
        pscr = ctx.enter_context(tc.tile_pool(name="pscr", bufs=4))
        qscr = ctx.enter_context(tc.tile_pool(name="qscr", bufs=6))
        wpool = ctx.enter_context(tc.tile_pool(name="wpool", bufs=10))
        ysb = ctx.enter_context(tc.tile_pool(name="ysb", bufs=3))
        bup = ctx.enter_context(tc.tile_pool(name="bup", bufs=6, space="PSUM"))
        yp = ctx.enter_context(tc.tile_pool(name="yp", bufs=2, space="PSUM"))

        # uT streamed in halves: cols [0:2048] then [2048:4096]
        u_t = [upool.tile([P, TSLAB], F16, tag=f"uT{k}", name=f"uT{k}")
               for k in range(KH)]
        cosb = [big.tile([P, L], F16, tag=f"cos{c}", name=f"cos{c}") for c in range(CCH)]
        sinb = [big.tile([P, L], F16, tag=f"sin{c}", name=f"sin{c}") for c in range(CCH)]
        v = [big.tile([P, L], F16, tag=f"v{j}", name=f"v{j}") for j in range(NCH)]
        bmat = [const.tile([P, 2 * N], F16, tag=f"B{k}", name=f"Bm{k}") for k in range(KH)]
        cmat = [const.tile([P, H], F16, tag=f"C{k}", name=f"Cm{k}") for k in range(NCH)]
        rdec_t = const.tile([P, CCH], F32, tag="rdec", name="rdec_t")

        # ---- constant DMAs (piece-A of u + Bcat first so Bu starts ASAP;
        #      table slab-0 pieces next for twiddle-in; Ccat last)
        nc.sync.dma_start(rdec_t[:], rdec[:])
        for k in range(KH):
            nc.sync.dma_start(bmat[k][:], Bcat[k * P:(k + 1) * P, :])
        # u piece A: t-cols [0:1024] + [3072:4096] -> tile cols [0:1024|1024:2048]
        for k in range(KH):
            nc.sync.dma_start(u_t[k][:, 0:SLAB], uT[k * P:(k + 1) * P, 0:SLAB])
            nc.sync.dma_start(u_t[k][:, SLAB:2 * SLAB],
                              uT[k * P:(k + 1) * P, 3 * SLAB:4 * SLAB])
        for c in range(CCH):
            nc.sync.dma_start(cosb[c][:, 0:SLAB], cosT[c * P:(c + 1) * P, 0:SLAB])
            nc.sync.dma_start(sinb[c][:, 0:SLAB], sinT[c * P:(c + 1) * P, 0:SLAB])
        for c in range(CCH):
            nc.sync.dma_start(cosb[c][:, SLAB:], cosT[c * P:(c + 1) * P, SLAB:])
            nc.sync.dma_start(sinb[c][:, SLAB:], sinT[c * P:(c + 1) * P, SLAB:])
        for k in range(NCH):
            nc.sync.dma_start(cmat[k][:], Ccat[k * P:(k + 1) * P, :])

        # ---- Phase A: Bu matmuls, evacuate into v slots (scan-time order) ----
        # u arrives in two pieces; piece A covers both directions' scan-slab 0.
        #   piece A: tsegs {0,1,7,6} at tile cols {0,512,1536,1024}
        #   piece B: tsegs {2,3,5,4} at tile cols {0,512,1536,1024}
        G_ORDER = [0, 4, 2, 6, 1, 5, 3, 7]
        UCOL = {0: 0, 1: 512, 7: 1536, 6: 1024, 2: 0, 3: 512, 5: 1536, 4: 1024}

        def do_group(nch, piece):
            tsegs = [0, 1, 7, 6] if piece == 0 else [2, 3, 5, 4]
            for ts in tsegs:
                ps = bup.tile([P, SEG], F32, name=f"bups{ts}", tag="bup")
                ucol = UCOL[ts]
                for k in range(KH):
                    nc.tensor.matmul(
                        ps[:],
                        bmat[k][:, nch * P:(nch + 1) * P],
                        u_t[k][:, ucol:ucol + SEG],
                        start=(k == 0), stop=(k == KH - 1),
                    )
                if _is_fwd_chunk(nch):
                    nc.scalar.copy(v[nch][:, ts * SEG:(ts + 1) * SEG], ps[:])
                else:
                    ss = NSEG - 1 - ts
                    dst = v[nch][:, ss * SEG:(ss + 1) * SEG]
                    nc.scalar.copy(dst[:, ::-1], ps[:])

        for nch in G_ORDER:
            do_group(nch, 0)
        # u piece B: t-cols [1024:2048] + [2048:3072]
        for k in range(KH):
            nc.sync.dma_start(u_t[k][:, 0:SLAB],
                              uT[k * P:(k + 1) * P, SLAB:2 * SLAB])
            nc.sync.dma_start(u_t[k][:, SLAB:2 * SLAB],
                              uT[k * P:(k + 1) * P, 2 * SLAB:3 * SLAB])
        for nch in G_ORDER:
            do_group(nch, 1)

        # ---- Phases B/C/D: twiddle-in + scan + untwiddle per (chunk, slab) ----
        prev_w = [None] * (2 * CCH)

        def twiddle_in(c, sb):
            jre, jim = c, c + CCH
            sl = slice(sb * SLAB, (sb + 1) * SLAB)
            cs, sn = cosb[c][:, sl], sinb[c][:, sl]
            vre, vim = v[jre][:, sl], v[jim][:, sl]
            p2 = pscr.tile([P, SLAB], F16, tag="p", name="p2")
            p3 = pscr.tile([P, SLAB], F16, tag="p", name="p3")
            p4 = pscr.tile([P, SLAB], F16, tag="p", name="p4")
            p1 = pscr.tile([P, SLAB], F16, tag="p", name="p1")
            nc.vector.tensor_mul(p2[:], sn, vim)
            nc.vector.tensor_mul(p4[:], sn, vre)
            nc.vector.tensor_mul(p3[:], cs, vim)
            nc.vector.tensor_mul(p1[:], cs, vre)
            nc.vector.tensor_add(vre, p1[:], p2[:])
            nc.vector.tensor_sub(vim, p3[:], p4[:])

        def scan_untw(c, sb):
            jre, jim = c, c + CCH
            sl = slice(sb * SLAB, (sb + 1) * SLAB)
            cs, sn = cosb[c][:, sl], sinb[c][:, sl]
            vre, vim = v[jre][:, sl], v[jim][:, sl]
            r_ap = rdec_t[:, c:c + 1].broadcast_to((P, SLAB))
            init_r = 0.0 if sb == 0 else prev_w[jre][:, SLAB - 1:SLAB]
            init_i = 0.0 if sb == 0 else prev_w[jim][:, SLAB - 1:SLAB]
            wr = wpool.tile([P, SLAB], F16, tag="w", name="wr")
            wi = wpool.tile([P, SLAB], F16, tag="w", name="wi")
            nc.vector.tensor_tensor_scan(wr[:], r_ap, vre, init_r,
                                         op0=MULT, op1=ADD)
            nc.vector.tensor_tensor_scan(wi[:], r_ap, vim, init_i,
                                         op0=MULT, op1=ADD)
            prev_w[jre], prev_w[jim] = wr, wi

            q1 = qscr.tile([P, SLAB], F16, tag="q", name="q1")
            q2 = qscr.tile([P, SLAB], F16, tag="q", name="q2")
            q4 = qscr.tile([P, SLAB], F16, tag="q", name="q4")
            q3 = qscr.tile([P, SLAB], F16, tag="q", name="q3")
            if c < 2:
                nc.vector.tensor_mul(q1[:], cs, wr[:])
                nc.vector.tensor_mul(q2[:], sn, wi[:])
                nc.vector.tensor_mul(q3[:], sn, wr[:])
                nc.vector.tensor_mul(q4[:], cs, wi[:])
                nc.vector.tensor_sub(vre, q1[:], q2[:])
                nc.vector.tensor_add(vim, q3[:], q4[:])
            else:
                # reversed READS flip scan order back to t order for free;
                # t-slab (NSLAB-1-sb) lands at table slab sb, t-ascending.
                nc.vector.tensor_mul(q1[:], cs[:, ::-1], wr[:, ::-1])
                nc.vector.tensor_mul(q2[:], sn[:, ::-1], wi[:, ::-1])
                nc.vector.tensor_mul(q3[:], sn[:, ::-1], wr[:, ::-1])
                nc.vector.tensor_mul(q4[:], cs[:, ::-1], wi[:, ::-1])
                nc.vector.tensor_sub(cs, q1[:], q2[:])
                nc.vector.tensor_add(sn, q3[:], q4[:])

        # x source for the output matmul: real chunk k, time block i.
        # bwd x: t-slab (i//8) lives at table slab (NSLAB-1-i//8), t-ordered.
        def x_src(k: int, i: int):
            if _is_fwd_chunk(k):
                return v[k][:, i * P:(i + 1) * P]
            c = k % 4
            col = (NSLAB - 1 - i // 8) * SLAB + (i % 8) * P
            src = cosb[c] if k < 4 else sinb[c]
            return src[:, col:col + P]

        def y_slab(sj):
            for o in range(SLAB // P):
                i = sj * (SLAB // P) + o
                py = yp.tile([P, H], F32, name="py", tag="yp")
                for k in range(NCH):
                    nc.tensor.matmul(
                        py[:], x_src(k, i), cmat[k][:],
                        start=(k == 0), stop=(k == NCH - 1),
                    )
                yt = ysb.tile([P, H], F32, tag="y", name="yt")
                nc.scalar.copy(yt[:], py[:])
                nc.sync.dma_start(y[i * P:(i + 1) * P, :], yt[:])

        # stream: slab-major, all 8 chains; y slabs interleave at readiness
        for c in C_ORDER:
            twiddle_in(c, 0)
        for c in C_ORDER:
            scan_untw(c, 0)
        for c in C_ORDER:
            twiddle_in(c, 1)
        for c in C_ORDER:
            scan_untw(c, 1)
        for c in C_ORDER:
            twiddle_in(c, 2)
        scan_untw(0, 2)
        scan_untw(1, 2)
        y_slab(2)            # fwd sb2 + bwd table-slab 1
        scan_untw(2, 2)
        scan_untw(3, 2)
        y_slab(1)            # fwd sb1 + bwd table-slab 2
        for c in C_ORDER:
            twiddle_in(c, 3)
        scan_untw(0, 3)
        scan_untw(1, 3)
        y_slab(3)            # fwd sb3 + bwd table-slab 0
        scan_untw(2, 3)
        scan_untw(3, 3)
        y_slab(0)            # fwd sb0 + bwd table-slab 3

    nc.compile()
    return nc


def prepare_inputs(u, lengths, nu_log, theta_log, B, C):
    """Host-side prep: per-core in_maps. All heavy math in fp64 for accuracy."""
    u = np.asarray(u)
    lengths = np.asarray(lengths)
    nu = np.exp(np.asarray(nu_log, np.float64))
    theta = np.exp(np.asarray(theta_log, np.float64))
    r = np.exp(-nu)                                    # |lam|, (N,)

    j = np.arange(L, dtype=np.float64)
    ang = np.mod(theta[:, None] * j[None, :], 2 * np.pi)   # (N, L)
    cos_base = np.cos(ang).astype(np.float16)
    sin_base = np.sin(ang).astype(np.float16)

    Bcat = np.empty((H, 2 * N), np.float16)
    Bcat[:, :N] = np.asarray(B)[..., 0]
    Bcat[:, N:] = np.asarray(B)[..., 1]
    Ccat = np.empty((2 * N, H), np.float16)
    Ccat[:N] = np.asarray(C)[0]
    Ccat[N:] = -np.asarray(C)[1]
    rdec = r.reshape(CCH, P).T.astype(np.float32).copy()   # (128, 4)

    half = N // 2
    in_maps = []
    for b in range(BSZ):
        ln = int(lengths[b])
        ub = np.array(u[b], np.float32)
        if ln < L:
            ub[ln:, :] = 0.0
        uTh = np.ascontiguousarray(ub.T.astype(np.float16))
        cosb = cos_base.copy()
        sinb = sin_base.copy()
        if ln < L:
            cosb[:half, ln:] = 0
            sinb[:half, ln:] = 0
            cosb[half:, :L - ln] = 0
            sinb[half:, :L - ln] = 0
        in_maps.append({
            "uT": uTh, "cosT": cosb, "sinT": sinb,
            "rdec": rdec, "Bcat": Bcat, "Ccat": Ccat,
        })
    return in_maps


def kernel(u, lengths, nu_log, theta_log, B, C):
    global _CACHED
    from concourse.bass_utils import run_bass_kernel_spmd
    in_maps = prepare_inputs(u, lengths, nu_log, theta_log, B, C)
    if _CACHED is None:
        _CACHED = build_nc()
    res = run_bass_kernel_spmd(_CACHED, in_maps, list(range(BSZ)))
    y = np.stack([res.results[i]["y"] for i in range(BSZ)], axis=0)
    return y.astype(np.float32)


# revision 23
# speedup vs baseline: 1.0279x; 1.0279x over previous
"""Bidirectional complex-diagonal LRU (Linear Recurrent Unit) on 8 Trainium2 cores.

Math: lam = exp(-exp(nu_log) + i*exp(theta_log)) per channel n (N=512).
  Bu = einsum('blh,hn->bnl', u, B0 + iB1), masked to length.
  Forward scan over channels [0,256), backward (time-reversed) over [256,512).
  y = x.real @ C0 - x.imag @ C1, zeroed past each sequence length.

Device strategy (data-parallel, one batch per core):
  - Rotation trick: x_t = e^{i*th*t} * w_t turns the complex recurrence
    x_t = lam x_{t-1} + Bu_t into TWO real recurrences w_t = r w_{t-1} + v_t
    (r = |lam|), each a native DVE tensor_tensor_scan along the free dim.
  - Twiddle tables cos/sin(th*j) built on host in fp64 (exact phases), fp16 on
    device. Per-core masking (zero columns past the sequence length) is folded
    into the tables, so masking costs nothing on device.
  - Backward channels run on the reversed time axis; reversal happens inside
    the PSUM-evacuation copy (negative-stride AP) and the untwiddle writes.
  - All matmuls fp16 (full PE rate), accumulation in fp32 PSUM. Scans run
    in place over the v buffer; x overwrites v (fwd) / spent cos+sin table
    slices (bwd), so SBUF holds everything with no extra big buffers.
  - Elementwise combine ops are split DVE/GPSIMD to balance engine load.

Self-contained: hardcodes B=8, L=4096, H=N=512, 8 cores.
"""

import numpy as np
from contextlib import ExitStack

import concourse.bass as bass
import concourse.bacc as bacc
import concourse.mybir as mybir
import concourse.tile as tile

P = 128
L = 4096
H = 512
N = 512
BSZ = 8
SEG = 512                # Bu matmul / evac granularity (one PSUM bank)
NSEG = L // SEG          # 8
SLAB = 1024              # scan + untwiddle granularity
NSLAB = L // SLAB        # 4
TSLAB = 2048             # twiddle-in granularity
NTSLAB = L // TSLAB      # 2
KH = H // P              # 4 contraction chunks for Bu
NCH = 2 * N // P         # 8 real-channel chunks (re 0..3, im 4..7)
CCH = N // P             # 4 complex-channel chunks (0,1 fwd; 2,3 bwd)
NT = L // P              # 32 time blocks for the output matmul

F16 = mybir.dt.float16
F32 = mybir.dt.float32
MULT = mybir.AluOpType.mult
ADD = mybir.AluOpType.add

C_ORDER = [0, 2, 1, 3]

_CACHED = None


def _is_fwd_chunk(nch: int) -> bool:
    return (nch % 4) < 2


def build_nc():
    nc = bacc.Bacc("TRN2", target_bir_lowering=False, debug=False)
    uT = nc.declare_dram_parameter("uT", [H, L], F16, isOutput=False)
    cosT = nc.declare_dram_parameter("cosT", [N, L], F16, isOutput=False)
    sinT = nc.declare_dram_parameter("sinT", [N, L], F16, isOutput=False)
    rdec = nc.declare_dram_parameter("rdec", [P, CCH], F32, isOutput=False)
    Bcat = nc.declare_dram_parameter("Bcat", [H, 2 * N], F16, isOutput=False)
    Ccat = nc.declare_dram_parameter("Ccat", [2 * N, H], F16, isOutput=False)
    y = nc.declare_dram_parameter("y", [L, H], F32, isOutput=True)

    with tile.TileContext(nc) as tc, ExitStack() as ctx:
        const = ctx.enter_context(tc.tile_pool(name="const", bufs=1))
        big = ctx.enter_context(tc.tile_pool(name="big", bufs=1))
        upool = ctx.enter_context(tc.tile_pool(name="upool", bufs=1))
        pscr = ctx.enter_context(tc.tile_pool(name="pscr", bufs=4))
        qscr = ctx.enter_context(tc.tile_pool(name="qscr", bufs=6))
        wpool = ctx.enter_context(tc.tile_pool(name="wpool", bufs=10))
        ysb = ctx.enter_context(tc.tile_pool(name="ysb", bufs=3))
        bup = ctx.enter_context(tc.tile_pool(name="bup", bufs=6, space="PSUM"))
        yp = ctx.enter_context(tc.tile_pool(name="yp", bufs=2, space="PSUM"))

        # uT streamed in halves: cols [0:2048] then [2048:4096]
        u_t = [upool.tile([P, TSLAB], F16, tag=f"uT{k}", name=f"uT{k}")
               for k in range(KH)]
        cosb = [big.tile([P, L], F16, tag=f"cos{c}", name=f"cos{c}") for c in range(CCH)]
        sinb = [big.tile([P, L], F16, tag=f"sin{c}", name=f"sin{c}") for c in range(CCH)]
        v = [big.tile([P, L], F16, tag=f"v{j}", name=f"v{j}") for j in range(NCH)]
        bmat = [const.tile([P, 2 * N], F16, tag=f"B{k}", name=f"Bm{k}") for k in range(KH)]
        cmat = [const.tile([P, H], F16, tag=f"C{k}", name=f"Cm{k}") for k in range(NCH)]
        rdec_t = const.tile([P, CCH], F32, tag="rdec", name="rdec_t")

        # ---- constant DMAs (piece-A of u + Bcat first so Bu starts ASAP;
        #      table slab-0 pieces next for twiddle-in; Ccat last)
        nc.sync.dma_start(rdec_t[:], rdec[:])
        for k in range(KH):
            nc.sync.dma_start(bmat[k][:], Bcat[k * P:(k + 1) * P, :])
        # u piece A: t-cols [0:1024] + [3072:4096] -> tile cols [0:1024|1024:2048]
        for k in range(KH):
            nc.sync.dma_start(u_t[k][:, 0:SLAB], uT[k * P:(k + 1) * P, 0:SLAB])
            nc.sync.dma_start(u_t[k][:, SLAB:2 * SLAB],
                              uT[k * P:(k + 1) * P, 3 * SLAB:4 * SLAB])
        for c in range(CCH):
            nc.sync.dma_start(cosb[c][:, 0:SLAB], cosT[c * P:(c + 1) * P, 0:SLAB])
            nc.sync.dma_start(sinb[c][:, 0:SLAB], sinT[c * P:(c + 1) * P, 0:SLAB])
        for c in range(CCH):
            nc.sync.dma_start(cosb[c][:, SLAB:], cosT[c * P:(c + 1) * P, SLAB:])
            nc.sync.dma_start(sinb[c][:, SLAB:], sinT[c * P:(c + 1) * P, SLAB:])
        for k in range(NCH):
            nc.sync.dma_start(cmat[k][:], Ccat[k * P:(k + 1) * P, :])

        # ---- Phase A: Bu matmuls, evacuate into v slots (scan-time order) ----
        # u arrives in two pieces; piece A covers both directions' scan-slab 0.
        #   piece A: tsegs {0,1,7,6} at tile cols {0,512,1536,1024}
        #   piece B: tsegs {2,3,5,4} at tile cols {0,512,1536,1024}
        G_ORDER = [0, 4, 2, 6, 1, 5, 3, 7]
        UCOL = {0: 0, 1: 512, 7: 1536, 6: 1024, 2: 0, 3: 512, 5: 1536, 4: 1024}

        def do_group(nch, piece):
            tsegs = [0, 1, 7, 6] if piece == 0 else [2, 3, 5, 4]
            for ts in tsegs:
                ps = bup.tile([P, SEG], F32, name=f"bups{ts}", tag="bup")
                ucol = UCOL[ts]
                for k in range(KH):
                    nc.tensor.matmul(
                        ps[:],
                        bmat[k][:, nch * P:(nch + 1) * P],
                        u_t[k][:, ucol:ucol + SEG],
                        start=(k == 0), stop=(k == KH - 1),
                    )
                if _is_fwd_chunk(nch):
                    nc.scalar.copy(v[nch][:, ts * SEG:(ts + 1) * SEG], ps[:])
                else:
                    ss = NSEG - 1 - ts
                    dst = v[nch][:, ss * SEG:(ss + 1) * SEG]
                    nc.scalar.copy(dst[:, ::-1], ps[:])

        for nch in G_ORDER:
            do_group(nch, 0)
        # u piece B: t-cols [1024:2048] + [2048:3072]
        for k in range(KH):
            nc.sync.dma_start(u_t[k][:, 0:SLAB],
                              uT[k * P:(k + 1) * P, SLAB:2 * SLAB])
            nc.sync.dma_start(u_t[k][:, SLAB:2 * SLAB],
                              uT[k * P:(k + 1) * P, 2 * SLAB:3 * SLAB])
        for nch in G_ORDER:
            do_group(nch, 1)

        # ---- Phases B/C/D: twiddle-in + scan + untwiddle per (chunk, slab) ----
        prev_w = [None] * (2 * CCH)

        def twiddle_in(c, sb):
            jre, jim = c, c + CCH
            sl = slice(sb * SLAB, (sb + 1) * SLAB)
            cs, sn = cosb[c][:, sl], sinb[c][:, sl]
            vre, vim = v[jre][:, sl], v[jim][:, sl]
            p2 = pscr.tile([P, SLAB], F16, tag="p", name="p2")
            p3 = pscr.tile([P, SLAB], F16, tag="p", name="p3")
            p4 = pscr.tile([P, SLAB], F16, tag="p", name="p4")
            p1 = pscr.tile([P, SLAB], F16, tag="p", name="p1")
            nc.vector.tensor_mul(p2[:], sn, vim)
            nc.vector.tensor_mul(p4[:], sn, vre)
            nc.vector.tensor_mul(p3[:], cs, vim)
            nc.vector.tensor_mul(p1[:], cs, vre)
            nc.vector.tensor_add(vre, p1[:], p2[:])
            nc.vector.tensor_sub(vim, p3[:], p4[:])

        def scan_untw(c, sb):
            jre, jim = c, c + CCH
            sl = slice(sb * SLAB, (sb + 1) * SLAB)
            cs, sn = cosb[c][:, sl], sinb[c][:, sl]
            vre, vim = v[jre][:, sl], v[jim][:, sl]
            r_ap = rdec_t[:, c:c + 1].broadcast_to((P, SLAB))
            init_r = 0.0 if sb == 0 else prev_w[jre][:, SLAB - 1:SLAB]
            init_i = 0.0 if sb == 0 else prev_w[jim][:, SLAB - 1:SLAB]
            wr = wpool.tile([P, SLAB], F16, tag="w", name="wr")
            wi = wpool.tile([P, SLAB], F16, tag="w", name="wi")
            nc.vector.tensor_tensor_scan(wr[:], r_ap, vre, init_r,
                                         op0=MULT, op1=ADD)
            nc.vector.tensor_tensor_scan(wi[:], r_ap, vim, init_i,
                                         op0=MULT, op1=ADD)
            prev_w[jre], prev_w[jim] = wr, wi

            q1 = qscr.tile([P, SLAB], F16, tag="q", name="q1")
            q2 = qscr.tile([P, SLAB], F16, tag="q", name="q2")
            q4 = qscr.tile([P, SLAB], F16, tag="q", name="q4")
            if c < 2:
                # xr = q1 - q2 (DVE); xi written as q3 into the spent v slab
                # then += q4 via SWDGE accum dma
                nc.vector.tensor_mul(q1[:], cs, wr[:])
                nc.vector.tensor_mul(q2[:], sn, wi[:])
                nc.vector.tensor_mul(vim, sn, wr[:])
                nc.vector.tensor_mul(q4[:], cs, wi[:])
                nc.vector.tensor_sub(vre, q1[:], q2[:])
                nc.gpsimd.dma_start(vim, q4[:], accum_op=mybir.AluOpType.add)
            else:
                # reversed READS flip scan order back to t order for free;
                # t-slab (NSLAB-1-sb) lands at table slab sb, t-ascending.
                nc.vector.tensor_mul(q1[:], cs[:, ::-1], wr[:, ::-1])
                nc.vector.tensor_mul(q2[:], sn[:, ::-1], wi[:, ::-1])
                nc.vector.tensor_mul(q4[:], sn[:, ::-1], wr[:, ::-1])
                nc.vector.tensor_mul(sn, cs[:, ::-1], wi[:, ::-1])
                nc.vector.tensor_sub(cs, q1[:], q2[:])
                nc.gpsimd.dma_start(sn, q4[:], accum_op=mybir.AluOpType.add)

        # x source for the output matmul: real chunk k, time block i.
        # bwd x: t-slab (i//8) lives at table slab (NSLAB-1-i//8), t-ordered.
        def x_src(k: int, i: int):
            if _is_fwd_chunk(k):
                return v[k][:, i * P:(i + 1) * P]
            c = k % 4
            col = (NSLAB - 1 - i // 8) * SLAB + (i % 8) * P
            src = cosb[c] if k < 4 else sinb[c]
            return src[:, col:col + P]

        def y_slab(sj):
            for o in range(SLAB // P):
                i = sj * (SLAB // P) + o
                py = yp.tile([P, H], F32, name="py", tag="yp")
                for k in range(NCH):
                    nc.tensor.matmul(
                        py[:], x_src(k, i), cmat[k][:],
                        start=(k == 0), stop=(k == NCH - 1),
                    )
                yt = ysb.tile([P, H], F32, tag="y", name="yt")
                nc.scalar.copy(yt[:], py[:])
                nc.sync.dma_start(y[i * P:(i + 1) * P, :], yt[:])

        # stream: slab-major, all 8 chains; y slabs interleave at readiness
        for c in C_ORDER:
            twiddle_in(c, 0)
        for c in C_ORDER:
            scan_untw(c, 0)
        for c in C_ORDER:
            twiddle_in(c, 1)
        for c in C_ORDER:
            scan_untw(c, 1)
        for c in C_ORDER:
            twiddle_in(c, 2)
        scan_untw(0, 2)
        scan_untw(1, 2)
        y_slab(2)            # fwd sb2 + bwd table-slab 1
        scan_untw(2, 2)
        scan_untw(3, 2)
        y_slab(1)            # fwd sb1 + bwd table-slab 2
        for c in C_ORDER:
            twiddle_in(c, 3)
        scan_untw(0, 3)
        scan_untw(1, 3)
        y_slab(3)            # fwd sb3 + bwd table-slab 0
        scan_untw(2, 3)
        scan_untw(3, 3)
        y_slab(0)            # fwd sb0 + bwd table-slab 3

    nc.compile()
    return nc


def prepare_inputs(u, lengths, nu_log, theta_log, B, C):
    """Host-side prep: per-core in_maps. All heavy math in fp64 for accuracy."""
    u = np.asarray(u)
    lengths = np.asarray(lengths)
    nu = np.exp(np.asarray(nu_log, np.float64))
    theta = np.exp(np.asarray(theta_log, np.float64))
    r = np.exp(-nu)                                    # |lam|, (N,)

    j = np.arange(L, dtype=np.float64)
    ang = np.mod(theta[:, None] * j[None, :], 2 * np.pi)   # (N, L)
    cos_base = np.cos(ang).astype(np.float16)
    sin_base = np.sin(ang).astype(np.float16)

    Bcat = np.empty((H, 2 * N), np.float16)
    Bcat[:, :N] = np.asarray(B)[..., 0]
    Bcat[:, N:] = np.asarray(B)[..., 1]
    Ccat = np.empty((2 * N, H), np.float16)
    Ccat[:N] = np.asarray(C)[0]
    Ccat[N:] = -np.asarray(C)[1]
    rdec = r.reshape(CCH, P).T.astype(np.float32).copy()   # (128, 4)

    half = N // 2
    in_maps = []
    for b in range(BSZ):
        ln = int(lengths[b])
        ub = np.array(u[b], np.float32)
        if ln < L:
            ub[ln:, :] = 0.0
        uTh = np.ascontiguousarray(ub.T.astype(np.float16))
        cosb = cos_base.copy()
        sinb = sin_base.copy()
        if ln < L:
            cosb[:half, ln:] = 0
            sinb[:half, ln:] = 0
            cosb[half:, :L - ln] = 0
            sinb[half:, :L - ln] = 0
        in_maps.append({
            "uT": uTh, "cosT": cosb, "sinT": sinb,
            "rdec": rdec, "Bcat": Bcat, "Ccat": Ccat,
        })
    return in_maps


def kernel(u, lengths, nu_log, theta_log, B, C):
    global _CACHED
    from concourse.bass_utils import run_bass_kernel_spmd
    in_maps = prepare_inputs(u, lengths, nu_log, theta_log, B, C)
    if _CACHED is None:
        _CACHED = build_nc()
    res = run_bass_kernel_spmd(_CACHED, in_maps, list(range(BSZ)))
    y = np.stack([res.results[i]["y"] for i in range(BSZ)], axis=0)
    return y.astype(np.float32)


# revision 25
# speedup vs baseline: 1.0291x; 1.0012x over previous
"""Bidirectional complex-diagonal LRU (Linear Recurrent Unit) on 8 Trainium2 cores.

Math: lam = exp(-exp(nu_log) + i*exp(theta_log)) per channel n (N=512).
  Bu = einsum('blh,hn->bnl', u, B0 + iB1), masked to length.
  Forward scan over channels [0,256), backward (time-reversed) over [256,512).
  y = x.real @ C0 - x.imag @ C1, zeroed past each sequence length.

Device strategy (data-parallel, one batch per core):
  - Rotation trick: x_t = e^{i*th*t} * w_t turns the complex recurrence
    x_t = lam x_{t-1} + Bu_t into TWO real recurrences w_t = r w_{t-1} + v_t
    (r = |lam|), each a native DVE tensor_tensor_scan along the free dim.
  - Twiddle tables cos/sin(th*j) built on host in fp64 (exact phases), fp16 on
    device. Per-core masking (zero columns past the sequence length) is folded
    into the tables, so masking costs nothing on device.
  - Backward channels run on the reversed time axis; reversal happens inside
    the PSUM-evacuation copy (negative-stride AP) and the untwiddle writes.
  - All matmuls fp16 (PE rate equals bf16), accumulation in fp32 PSUM.
    x overwrites the spent v slabs (fwd) / cos+sin table slabs (bwd), so
    SBUF holds everything with no extra big buffers.
  - u streams in two pieces (outer quarters first) so both scan directions
    start immediately; untwiddle ADD-combines ride SWDGE accumulate DMAs.

Self-contained: hardcodes B=8, L=4096, H=N=512, 8 cores.
"""

import numpy as np
from contextlib import ExitStack

import concourse.bass as bass
import concourse.bacc as bacc
import concourse.mybir as mybir
import concourse.tile as tile

P = 128
L = 4096
H = 512
N = 512
BSZ = 8
SEG = 512                # Bu matmul / evac granularity (one PSUM bank)
NSEG = L // SEG          # 8
SLAB = 1024              # scan + untwiddle granularity
NSLAB = L // SLAB        # 4
TSLAB = 2048             # u-tile width (u streams in 2 pieces)
NTSLAB = L // TSLAB      # 2
KH = H // P              # 4 contraction chunks for Bu
NCH = 2 * N // P         # 8 real-channel chunks (re 0..3, im 4..7)
CCH = N // P             # 4 complex-channel chunks (0,1 fwd; 2,3 bwd)
NT = L // P              # 32 time blocks for the output matmul

F16 = mybir.dt.float16
F32 = mybir.dt.float32
MULT = mybir.AluOpType.mult
ADD = mybir.AluOpType.add

C_ORDER = [0, 2, 1, 3]

_CACHED = None


def _is_fwd_chunk(nch: int) -> bool:
    return (nch % 4) < 2


def build_nc():
    nc = bacc.Bacc("TRN2", target_bir_lowering=False, debug=False)
    uT = nc.declare_dram_parameter("uT", [H, L], F16, isOutput=False)
    cosT = nc.declare_dram_parameter("cosT", [N, L], F16, isOutput=False)
    sinT = nc.declare_dram_parameter("sinT", [N, L], F16, isOutput=False)
    rdec = nc.declare_dram_parameter("rdec", [P, CCH], F32, isOutput=False)
    Bcat = nc.declare_dram_parameter("Bcat", [H, 2 * N], F16, isOutput=False)
    Ccat = nc.declare_dram_parameter("Ccat", [2 * N, H], F16, isOutput=False)
    y = nc.declare_dram_parameter("y", [L, H], F32, isOutput=True)

    with tile.TileContext(nc) as tc, ExitStack() as ctx:
        const = ctx.enter_context(tc.tile_pool(name="const", bufs=1))
        big = ctx.enter_context(tc.tile_pool(name="big", bufs=1))
        upool = ctx.enter_context(tc.tile_pool(name="upool", bufs=1))
        pscr = ctx.enter_context(tc.tile_pool(name="pscr", bufs=4))
        qscr = ctx.enter_context(tc.tile_pool(name="qscr", bufs=6))
        wpool = ctx.enter_context(tc.tile_pool(name="wpool", bufs=10))
        ysb = ctx.enter_context(tc.tile_pool(name="ysb", bufs=3))
        bup = ctx.enter_context(tc.tile_pool(name="bup", bufs=6, space="PSUM"))
        yp = ctx.enter_context(tc.tile_pool(name="yp", bufs=2, space="PSUM"))

        # uT streamed in halves: cols [0:2048] then [2048:4096]
        u_t = [upool.tile([P, TSLAB], F16, tag=f"uT{k}", name=f"uT{k}")
               for k in range(KH)]
        cosb = [big.tile([P, L], F16, tag=f"cos{c}", name=f"cos{c}") for c in range(CCH)]
        sinb = [big.tile([P, L], F16, tag=f"sin{c}", name=f"sin{c}") for c in range(CCH)]
        v = [big.tile([P, L], F16, tag=f"v{j}", name=f"v{j}") for j in range(NCH)]
        bmat = [const.tile([P, 2 * N], F16, tag=f"B{k}", name=f"Bm{k}") for k in range(KH)]
        cmat = [const.tile([P, H], F16, tag=f"C{k}", name=f"Cm{k}") for k in range(NCH)]
        rdec_t = const.tile([P, CCH], F32, tag="rdec", name="rdec_t")

        # ---- constant DMAs (piece-A of u + Bcat first so Bu starts ASAP;
        #      table slab-0 pieces next for twiddle-in; Ccat last)
        nc.sync.dma_start(rdec_t[:], rdec[:])
        for k in range(KH):
            nc.sync.dma_start(bmat[k][:], Bcat[k * P:(k + 1) * P, :])
        # u piece A: t-cols [0:1024] + [3072:4096] -> tile cols [0:1024|1024:2048]
        for k in range(KH):
            nc.sync.dma_start(u_t[k][:, 0:SLAB], uT[k * P:(k + 1) * P, 0:SLAB])
            nc.sync.dma_start(u_t[k][:, SLAB:2 * SLAB],
                              uT[k * P:(k + 1) * P, 3 * SLAB:4 * SLAB])
        for c in range(CCH):
            nc.sync.dma_start(cosb[c][:, 0:SLAB], cosT[c * P:(c + 1) * P, 0:SLAB])
            nc.sync.dma_start(sinb[c][:, 0:SLAB], sinT[c * P:(c + 1) * P, 0:SLAB])
        for c in range(CCH):
            nc.sync.dma_start(cosb[c][:, SLAB:], cosT[c * P:(c + 1) * P, SLAB:])
            nc.sync.dma_start(sinb[c][:, SLAB:], sinT[c * P:(c + 1) * P, SLAB:])
        for k in range(NCH):
            nc.sync.dma_start(cmat[k][:], Ccat[k * P:(k + 1) * P, :])

        # ---- Phase A: Bu matmuls, evacuate into v slots (scan-time order) ----
        # u arrives in two pieces; piece A covers both directions' scan-slab 0.
        #   piece A: tsegs {0,1,7,6} at tile cols {0,512,1536,1024}
        #   piece B: tsegs {2,3,5,4} at tile cols {0,512,1536,1024}
        G_ORDER = [0, 4, 2, 6, 1, 5, 3, 7]
        UCOL = {0: 0, 1: 512, 7: 1536, 6: 1024, 2: 0, 3: 512, 5: 1536, 4: 1024}

        def do_group(nch, piece):
            tsegs = [0, 1, 7, 6] if piece == 0 else [2, 3, 5, 4]
            for ts in tsegs:
                ps = bup.tile([P, SEG], F32, name=f"bups{ts}", tag="bup")
                ucol = UCOL[ts]
                for k in range(KH):
                    nc.tensor.matmul(
                        ps[:],
                        bmat[k][:, nch * P:(nch + 1) * P],
                        u_t[k][:, ucol:ucol + SEG],
                        start=(k == 0), stop=(k == KH - 1),
                    )
                if _is_fwd_chunk(nch):
                    nc.scalar.copy(v[nch][:, ts * SEG:(ts + 1) * SEG], ps[:])
                else:
                    ss = NSEG - 1 - ts
                    dst = v[nch][:, ss * SEG:(ss + 1) * SEG]
                    nc.scalar.copy(dst[:, ::-1], ps[:])

        for nch in G_ORDER:
            do_group(nch, 0)
        # u piece B: t-cols [1024:2048] + [2048:3072]
        for k in range(KH):
            nc.sync.dma_start(u_t[k][:, 0:SLAB],
                              uT[k * P:(k + 1) * P, SLAB:2 * SLAB])
            nc.sync.dma_start(u_t[k][:, SLAB:2 * SLAB],
                              uT[k * P:(k + 1) * P, 2 * SLAB:3 * SLAB])
        for nch in G_ORDER:
            do_group(nch, 1)

        # ---- Phases B/C/D: twiddle-in + scan + untwiddle per (chunk, slab) ----
        prev_w = [None] * (2 * CCH)

        def twiddle_in(c, sb):
            jre, jim = c, c + CCH
            sl = slice(sb * SLAB, (sb + 1) * SLAB)
            cs, sn = cosb[c][:, sl], sinb[c][:, sl]
            vre, vim = v[jre][:, sl], v[jim][:, sl]
            p2 = pscr.tile([P, SLAB], F16, tag="p", name="p2")
            p3 = pscr.tile([P, SLAB], F16, tag="p", name="p3")
            p4 = pscr.tile([P, SLAB], F16, tag="p", name="p4")
            p1 = pscr.tile([P, SLAB], F16, tag="p", name="p1")
            nc.vector.tensor_mul(p2[:], sn, vim)
            nc.vector.tensor_mul(p4[:], sn, vre)
            nc.vector.tensor_mul(p3[:], cs, vim)
            nc.vector.tensor_mul(p1[:], cs, vre)
            nc.vector.tensor_add(vre, p1[:], p2[:])
            nc.vector.tensor_sub(vim, p3[:], p4[:])

        def scan_untw(c, sb):
            jre, jim = c, c + CCH
            sl = slice(sb * SLAB, (sb + 1) * SLAB)
            cs, sn = cosb[c][:, sl], sinb[c][:, sl]
            vre, vim = v[jre][:, sl], v[jim][:, sl]
            r_ap = rdec_t[:, c:c + 1].broadcast_to((P, SLAB))
            init_r = 0.0 if sb == 0 else prev_w[jre][:, SLAB - 1:SLAB]
            init_i = 0.0 if sb == 0 else prev_w[jim][:, SLAB - 1:SLAB]
            wr = wpool.tile([P, SLAB], F16, tag="w", name="wr")
            wi = wpool.tile([P, SLAB], F16, tag="w", name="wi")
            nc.vector.tensor_tensor_scan(wr[:], r_ap, vre, init_r,
                                         op0=MULT, op1=ADD)
            nc.vector.tensor_tensor_scan(wi[:], r_ap, vim, init_i,
                                         op0=MULT, op1=ADD)
            prev_w[jre], prev_w[jim] = wr, wi

            q1 = qscr.tile([P, SLAB], F16, tag="q", name="q1")
            q2 = qscr.tile([P, SLAB], F16, tag="q", name="q2")
            q4 = qscr.tile([P, SLAB], F16, tag="q", name="q4")
            if c < 2:
                # xr = q1 - q2 (DVE); xi written as q3 into the spent v slab
                # then += q4 via SWDGE accum dma
                nc.vector.tensor_mul(q1[:], cs, wr[:])
                nc.vector.tensor_mul(q2[:], sn, wi[:])
                nc.vector.tensor_mul(vim, sn, wr[:])
                nc.vector.tensor_mul(q4[:], cs, wi[:])
                nc.vector.tensor_sub(vre, q1[:], q2[:])
                nc.gpsimd.dma_start(vim, q4[:], accum_op=mybir.AluOpType.add)
            else:
                # reversed READS flip scan order back to t order for free;
                # t-slab (NSLAB-1-sb) lands at table slab sb, t-ascending.
                nc.vector.tensor_mul(q1[:], cs[:, ::-1], wr[:, ::-1])
                nc.vector.tensor_mul(q2[:], sn[:, ::-1], wi[:, ::-1])
                nc.vector.tensor_mul(q4[:], sn[:, ::-1], wr[:, ::-1])
                nc.vector.tensor_mul(sn, cs[:, ::-1], wi[:, ::-1])
                nc.vector.tensor_sub(cs, q1[:], q2[:])
                nc.gpsimd.dma_start(sn, q4[:], accum_op=mybir.AluOpType.add)

        # x source for the output matmul: real chunk k, time block i.
        # bwd x: t-slab (i//8) lives at table slab (NSLAB-1-i//8), t-ordered.
        def x_src(k: int, i: int):
            if _is_fwd_chunk(k):
                return v[k][:, i * P:(i + 1) * P]
            c = k % 4
            col = (NSLAB - 1 - i // 8) * SLAB + (i % 8) * P
            src = cosb[c] if k < 4 else sinb[c]
            return src[:, col:col + P]

        def y_slab(sj):
            for o in range(SLAB // P):
                i = sj * (SLAB // P) + o
                py = yp.tile([P, H], F32, name="py", tag="yp")
                for k in range(NCH):
                    nc.tensor.matmul(
                        py[:], x_src(k, i), cmat[k][:],
                        start=(k == 0), stop=(k == NCH - 1),
                    )
                yt = ysb.tile([P, H], F32, tag="y", name="yt")
                nc.scalar.copy(yt[:], py[:])
                nc.sync.dma_start(y[i * P:(i + 1) * P, :], yt[:])

        # stream: slab-major, all 8 chains; y slabs interleave at readiness
        for c in C_ORDER:
            twiddle_in(c, 0)
        for c in C_ORDER:
            scan_untw(c, 0)
        for c in C_ORDER:
            twiddle_in(c, 1)
        for c in C_ORDER:
            scan_untw(c, 1)
        # run the forward chains to completion first: Y2 and Y3 unblock as
        # soon as fwd slabs 2,3 land (their bwd deps finished in slabs 0,1);
        # then trail the backward chains for Y1, Y0.
        twiddle_in(0, 2)
        twiddle_in(1, 2)
        scan_untw(0, 2)
        scan_untw(1, 2)
        y_slab(2)            # fwd sb2 + bwd table-slab 1
        twiddle_in(0, 3)
        twiddle_in(1, 3)
        scan_untw(0, 3)
        scan_untw(1, 3)
        y_slab(3)            # fwd sb3 + bwd table-slab 0
        twiddle_in(2, 2)
        twiddle_in(3, 2)
        scan_untw(2, 2)
        scan_untw(3, 2)
        y_slab(1)            # fwd sb1 + bwd table-slab 2
        twiddle_in(2, 3)
        twiddle_in(3, 3)
        scan_untw(2, 3)
        scan_untw(3, 3)
        y_slab(0)            # fwd sb0 + bwd table-slab 3

    nc.compile()
    return nc


def prepare_inputs(u, lengths, nu_log, theta_log, B, C):
    """Host-side prep: per-core in_maps. All heavy math in fp64 for accuracy."""
    u = np.asarray(u)
    lengths = np.asarray(lengths)
    nu = np.exp(np.asarray(nu_log, np.float64))
    theta = np.exp(np.asarray(theta_log, np.float64))
    r = np.exp(-nu)                                    # |lam|, (N,)

    j = np.arange(L, dtype=np.float64)
    ang = np.mod(theta[:, None] * j[None, :], 2 * np.pi)   # (N, L)
    cos_base = np.cos(ang).astype(np.float16)
    sin_base = np.sin(ang).astype(np.float16)

    Bcat = np.empty((H, 2 * N), np.float16)
    Bcat[:, :N] = np.asarray(B)[..., 0]
    Bcat[:, N:] = np.asarray(B)[..., 1]
    Ccat = np.empty((2 * N, H), np.float16)
    Ccat[:N] = np.asarray(C)[0]
    Ccat[N:] = -np.asarray(C)[1]
    rdec = r.reshape(CCH, P).T.astype(np.float32).copy()   # (128, 4)

    half = N // 2
    in_maps = []
    for b in range(BSZ):
        ln = int(lengths[b])
        ub = np.array(u[b], np.float32)
        if ln < L:
            ub[ln:, :] = 0.0
        uTh = np.ascontiguousarray(ub.T.astype(np.float16))
        cosb = cos_base.copy()
        sinb = sin_base.copy()
        if ln < L:
            cosb[:half, ln:] = 0
            sinb[:half, ln:] = 0
            cosb[half:, :L - ln] = 0
            sinb[half:, :L - ln] = 0
        in_maps.append({
            "uT": uTh, "cosT": cosb, "sinT": sinb,
            "rdec": rdec, "Bcat": Bcat, "Ccat": Ccat,
        })
    return in_maps


def kernel(u, lengths, nu_log, theta_log, B, C):
    global _CACHED
    from concourse.bass_utils import run_bass_kernel_spmd
    in_maps = prepare_inputs(u, lengths, nu_log, theta_log, B, C)
    if _CACHED is None:
        _CACHED = build_nc()
    res = run_bass_kernel_spmd(_CACHED, in_maps, list(range(BSZ)))
    y = np.stack([res.results[i]["y"] for i in range(BSZ)], axis=0)
    return y.astype(np.float32)


# revision 26
# speedup vs baseline: 1.0470x; 1.0174x over previous
"""Bidirectional complex-diagonal LRU (Linear Recurrent Unit) on 8 Trainium2 cores.

Math: lam = exp(-exp(nu_log) + i*exp(theta_log)) per channel n (N=512).
  Bu = einsum('blh,hn->bnl', u, B0 + iB1), masked to length.
  Forward scan over channels [0,256), backward (time-reversed) over [256,512).
  y = x.real @ C0 - x.imag @ C1, zeroed past each sequence length.

Device strategy (data-parallel, one batch per core):
  - Rotation trick: x_t = e^{i*th*t} * w_t turns the complex recurrence
    x_t = lam x_{t-1} + Bu_t into TWO real recurrences w_t = r w_{t-1} + v_t
    (r = |lam|), each a native DVE tensor_tensor_scan along the free dim.
  - Twiddle tables cos/sin(th*j) built on host in fp64 (exact phases), fp16 on
    device. Per-core masking (zero columns past the sequence length) is folded
    into the tables, so masking costs nothing on device.
  - Backward channels run on the reversed time axis; reversal happens inside
    the PSUM-evacuation copy (negative-stride AP) and the untwiddle writes.
  - All matmuls fp16 (PE rate equals bf16), accumulation in fp32 PSUM.
    x overwrites the spent v slabs (fwd) / cos+sin table slabs (bwd), so
    SBUF holds everything with no extra big buffers.
  - u streams in two pieces (outer quarters first) so both scan directions
    start immediately; untwiddle ADD-combines ride SWDGE accumulate DMAs.

Self-contained: hardcodes B=8, L=4096, H=N=512, 8 cores.
"""

import numpy as np
from contextlib import ExitStack

import concourse.bass as bass
import concourse.bacc as bacc
import concourse.mybir as mybir
import concourse.tile as tile

P = 128
L = 4096
H = 512
N = 512
BSZ = 8
SEG = 512                # Bu matmul / evac granularity (one PSUM bank)
NSEG = L // SEG          # 8
SLAB = 1024              # scan + untwiddle granularity
NSLAB = L // SLAB        # 4
TSLAB = 2048             # u-tile width (u streams in 2 pieces)
NTSLAB = L // TSLAB      # 2
KH = H // P              # 4 contraction chunks for Bu
NCH = 2 * N // P         # 8 real-channel chunks (re 0..3, im 4..7)
CCH = N // P             # 4 complex-channel chunks (0,1 fwd; 2,3 bwd)
NT = L // P              # 32 time blocks for the output matmul

F16 = mybir.dt.float16
F32 = mybir.dt.float32
MULT = mybir.AluOpType.mult
ADD = mybir.AluOpType.add

C_ORDER = [0, 2, 1, 3]

_CACHED = None


def _is_fwd_chunk(nch: int) -> bool:
    return (nch % 4) < 2


def build_nc():
    nc = bacc.Bacc("TRN2", target_bir_lowering=False, debug=False)
    uT = nc.declare_dram_parameter("uT", [H, L], F16, isOutput=False)
    cosT = nc.declare_dram_parameter("cosT", [N, L], F16, isOutput=False)
    sinT = nc.declare_dram_parameter("sinT", [N, L], F16, isOutput=False)
    rdec = nc.declare_dram_parameter("rdec", [P, CCH], F32, isOutput=False)
    Bcat = nc.declare_dram_parameter("Bcat", [H, 2 * N], F16, isOutput=False)
    Ccat = nc.declare_dram_parameter("Ccat", [2 * N, H], F16, isOutput=False)
    y = nc.declare_dram_parameter("y", [L, H], F32, isOutput=True)

    with tile.TileContext(nc) as tc, ExitStack() as ctx:
        const = ctx.enter_context(tc.tile_pool(name="const", bufs=1))
        big = ctx.enter_context(tc.tile_pool(name="big", bufs=1))
        upool = ctx.enter_context(tc.tile_pool(name="upool", bufs=1))
        pscr = ctx.enter_context(tc.tile_pool(name="pscr", bufs=4))
        qscr = ctx.enter_context(tc.tile_pool(name="qscr", bufs=6))
        wpool = ctx.enter_context(tc.tile_pool(name="wpool", bufs=10))
        ysb = ctx.enter_context(tc.tile_pool(name="ysb", bufs=3))
        bup = ctx.enter_context(tc.tile_pool(name="bup", bufs=6, space="PSUM"))
        yp = ctx.enter_context(tc.tile_pool(name="yp", bufs=2, space="PSUM"))

        # uT streamed in halves: cols [0:2048] then [2048:4096]
        u_t = [upool.tile([P, TSLAB], F16, tag=f"uT{k}", name=f"uT{k}")
               for k in range(KH)]
        cosb = [big.tile([P, L], F16, tag=f"cos{c}", name=f"cos{c}") for c in range(CCH)]
        sinb = [big.tile([P, L], F16, tag=f"sin{c}", name=f"sin{c}") for c in range(CCH)]
        v = [big.tile([P, L], F16, tag=f"v{j}", name=f"v{j}") for j in range(NCH)]
        bmat = [const.tile([P, 2 * N], F16, tag=f"B{k}", name=f"Bm{k}") for k in range(KH)]
        cmat = [const.tile([P, H], F16, tag=f"C{k}", name=f"Cm{k}") for k in range(NCH)]
        rdec_t = const.tile([P, CCH], F32, tag="rdec", name="rdec_t")

        # ---- constant DMAs (piece-A of u + Bcat first so Bu starts ASAP;
        #      table slab-0 pieces next for twiddle-in; Ccat last)
        nc.sync.dma_start(rdec_t[:], rdec[:])
        for k in range(KH):
            nc.sync.dma_start(bmat[k][:], Bcat[k * P:(k + 1) * P, :])
        # u piece A: t-cols [0:1024] + [3072:4096] -> tile cols [0:1024|1024:2048]
        for k in range(KH):
            nc.sync.dma_start(u_t[k][:, 0:SLAB], uT[k * P:(k + 1) * P, 0:SLAB])
            nc.sync.dma_start(u_t[k][:, SLAB:2 * SLAB],
                              uT[k * P:(k + 1) * P, 3 * SLAB:4 * SLAB])
        for c in range(CCH):
            nc.sync.dma_start(cosb[c][:, 0:SLAB], cosT[c * P:(c + 1) * P, 0:SLAB])
            nc.sync.dma_start(sinb[c][:, 0:SLAB], sinT[c * P:(c + 1) * P, 0:SLAB])
        for c in range(CCH):
            nc.sync.dma_start(cosb[c][:, SLAB:], cosT[c * P:(c + 1) * P, SLAB:])
            nc.sync.dma_start(sinb[c][:, SLAB:], sinT[c * P:(c + 1) * P, SLAB:])
        for k in range(NCH):
            nc.sync.dma_start(cmat[k][:], Ccat[k * P:(k + 1) * P, :])

        # ---- Phase A: Bu matmuls, evacuate into v slots (scan-time order) ----
        # u arrives in two pieces; piece A covers both directions' scan-slab 0.
        #   piece A: tsegs {0,1,7,6} at tile cols {0,512,1536,1024}
        #   piece B: tsegs {2,3,5,4} at tile cols {0,512,1536,1024}
        G_ORDER = [0, 4, 2, 6, 1, 5, 3, 7]
        UCOL = {0: 0, 1: 512, 7: 1536, 6: 1024, 2: 0, 3: 512, 5: 1536, 4: 1024}

        def do_group(nch, piece):
            tsegs = [0, 1, 7, 6] if piece == 0 else [2, 3, 5, 4]
            for ts in tsegs:
                ps = bup.tile([P, SEG], F32, name=f"bups{ts}", tag="bup")
                ucol = UCOL[ts]
                for k in range(KH):
                    nc.tensor.matmul(
                        ps[:],
                        bmat[k][:, nch * P:(nch + 1) * P],
                        u_t[k][:, ucol:ucol + SEG],
                        start=(k == 0), stop=(k == KH - 1),
                    )
                if _is_fwd_chunk(nch):
                    nc.scalar.copy(v[nch][:, ts * SEG:(ts + 1) * SEG], ps[:])
                else:
                    ss = NSEG - 1 - ts
                    dst = v[nch][:, ss * SEG:(ss + 1) * SEG]
                    nc.scalar.copy(dst[:, ::-1], ps[:])

        for nch in G_ORDER:
            do_group(nch, 0)
        # u piece B: t-cols [1024:2048] + [2048:3072]
        for k in range(KH):
            nc.sync.dma_start(u_t[k][:, 0:SLAB],
                              uT[k * P:(k + 1) * P, SLAB:2 * SLAB])
            nc.sync.dma_start(u_t[k][:, SLAB:2 * SLAB],
                              uT[k * P:(k + 1) * P, 2 * SLAB:3 * SLAB])
        for nch in G_ORDER:
            do_group(nch, 1)

        # ---- Phases B/C/D: twiddle-in + scan + untwiddle per (chunk, slab) ----
        prev_w = [None] * (2 * CCH)

        def twiddle_in(c, sb):
            jre, jim = c, c + CCH
            sl = slice(sb * SLAB, (sb + 1) * SLAB)
            cs, sn = cosb[c][:, sl], sinb[c][:, sl]
            vre, vim = v[jre][:, sl], v[jim][:, sl]
            p2 = pscr.tile([P, SLAB], F16, tag="p", name="p2")
            p3 = pscr.tile([P, SLAB], F16, tag="p", name="p3")
            p4 = pscr.tile([P, SLAB], F16, tag="p", name="p4")
            p1 = pscr.tile([P, SLAB], F16, tag="p", name="p1")
            nc.vector.tensor_mul(p2[:], sn, vim)
            nc.vector.tensor_mul(p4[:], sn, vre)
            nc.vector.tensor_mul(p3[:], cs, vim)
            nc.vector.tensor_mul(p1[:], cs, vre)
            nc.vector.tensor_add(vre, p1[:], p2[:])
            nc.vector.tensor_sub(vim, p3[:], p4[:])

        def scan_untw(c, sb):
            jre, jim = c, c + CCH
            sl = slice(sb * SLAB, (sb + 1) * SLAB)
            cs, sn = cosb[c][:, sl], sinb[c][:, sl]
            vre, vim = v[jre][:, sl], v[jim][:, sl]
            r_ap = rdec_t[:, c:c + 1].broadcast_to((P, SLAB))
            init_r = 0.0 if sb == 0 else prev_w[jre][:, SLAB - 1:SLAB]
            init_i = 0.0 if sb == 0 else prev_w[jim][:, SLAB - 1:SLAB]
            wr = wpool.tile([P, SLAB], F16, tag="w", name="wr")
            wi = wpool.tile([P, SLAB], F16, tag="w", name="wi")
            nc.vector.tensor_tensor_scan(wr[:], r_ap, vre, init_r,
                                         op0=MULT, op1=ADD)
            nc.vector.tensor_tensor_scan(wi[:], r_ap, vim, init_i,
                                         op0=MULT, op1=ADD)
            prev_w[jre], prev_w[jim] = wr, wi

            q1 = qscr.tile([P, SLAB], F16, tag="q", name="q1")
            q2 = qscr.tile([P, SLAB], F16, tag="q", name="q2")
            q4 = qscr.tile([P, SLAB], F16, tag="q", name="q4")
            if c < 2:
                # xr = q1 - q2 (DVE); xi written as q3 into the spent v slab
                # then += q4 via SWDGE accum dma
                nc.vector.tensor_mul(q1[:], cs, wr[:])
                nc.vector.tensor_mul(q2[:], sn, wi[:])
                nc.vector.tensor_mul(vim, sn, wr[:])
                nc.vector.tensor_mul(q4[:], cs, wi[:])
                nc.vector.tensor_sub(vre, q1[:], q2[:])
                nc.gpsimd.dma_start(vim, q4[:], accum_op=mybir.AluOpType.add)
            else:
                # reversed READS flip scan order back to t order for free;
                # t-slab (NSLAB-1-sb) lands at table slab sb, t-ascending.
                nc.vector.tensor_mul(q1[:], cs[:, ::-1], wr[:, ::-1])
                nc.vector.tensor_mul(q2[:], sn[:, ::-1], wi[:, ::-1])
                nc.vector.tensor_mul(q4[:], sn[:, ::-1], wr[:, ::-1])
                nc.vector.tensor_mul(sn, cs[:, ::-1], wi[:, ::-1])
                nc.vector.tensor_sub(cs, q1[:], q2[:])
                nc.gpsimd.dma_start(sn, q4[:], accum_op=mybir.AluOpType.add)

        # x source for the output matmul: real chunk k, time block i.
        # bwd x: t-slab (i//8) lives at table slab (NSLAB-1-i//8), t-ordered.
        def x_src(k: int, i: int):
            if _is_fwd_chunk(k):
                return v[k][:, i * P:(i + 1) * P]
            c = k % 4
            col = (NSLAB - 1 - i // 8) * SLAB + (i % 8) * P
            src = cosb[c] if k < 4 else sinb[c]
            return src[:, col:col + P]

        def y_slab(sj):
            for o in range(SLAB // P):
                i = sj * (SLAB // P) + o
                py = yp.tile([P, H], F32, name="py", tag="yp")
                for k in range(NCH):
                    nc.tensor.matmul(
                        py[:], x_src(k, i), cmat[k][:],
                        start=(k == 0), stop=(k == NCH - 1),
                    )
                yt = ysb.tile([P, H], F32, tag="y", name="yt")
                nc.scalar.copy(yt[:], py[:])
                nc.sync.dma_start(y[i * P:(i + 1) * P, :], yt[:])

        # split emission: fwd-chunk contributions first (their x is ready
        # early), bwd completes after the last backward scans; the 8 PSUM
        # banks (6 from the idle Bu pool + 2 from yp) hold in between.
        def y_slab_fwd(sj):
            tiles = []
            for o in range(SLAB // P):
                i = sj * (SLAB // P) + o
                pool = bup if o < 6 else yp
                tag = "bup" if o < 6 else "yp"
                py = pool.tile([P, H], F32, name="pys", tag=tag)
                first = True
                for k in (0, 1, 4, 5):
                    nc.tensor.matmul(py[:], x_src(k, i), cmat[k][:],
                                     start=first, stop=False)
                    first = False
                tiles.append(py)
            return tiles

        def y_slab_bwd(sj, tiles):
            for o in range(SLAB // P):
                i = sj * (SLAB // P) + o
                py = tiles[o]
                for k in (2, 3, 6, 7):
                    nc.tensor.matmul(py[:], x_src(k, i), cmat[k][:],
                                     start=False, stop=(k == 7))
                yt = ysb.tile([P, H], F32, tag="y", name="yt")
                nc.scalar.copy(yt[:], py[:])
                nc.sync.dma_start(y[i * P:(i + 1) * P, :], yt[:])

        # stream: slab-major, all 8 chains; y slabs interleave at readiness
        for c in C_ORDER:
            twiddle_in(c, 0)
        for c in C_ORDER:
            scan_untw(c, 0)
        for c in C_ORDER:
            twiddle_in(c, 1)
        for c in C_ORDER:
            scan_untw(c, 1)
        # run the forward chains to completion first: Y2 and Y3 unblock as
        # soon as fwd slabs 2,3 land (their bwd deps finished in slabs 0,1);
        # then trail the backward chains for Y1, Y0.
        twiddle_in(0, 2)
        twiddle_in(1, 2)
        scan_untw(0, 2)
        scan_untw(1, 2)
        y_slab(2)            # fwd sb2 + bwd table-slab 1
        twiddle_in(0, 3)
        twiddle_in(1, 3)
        scan_untw(0, 3)
        scan_untw(1, 3)
        y_slab(3)            # fwd sb3 + bwd table-slab 0
        t1 = y_slab_fwd(1)
        twiddle_in(2, 2)
        twiddle_in(3, 2)
        scan_untw(2, 2)
        scan_untw(3, 2)
        y_slab_bwd(1, t1)    # + bwd table-slab 2
        t0_ = y_slab_fwd(0)
        twiddle_in(2, 3)
        twiddle_in(3, 3)
        scan_untw(2, 3)
        scan_untw(3, 3)
        y_slab_bwd(0, t0_)   # + bwd table-slab 3

    nc.compile()
    return nc


def prepare_inputs(u, lengths, nu_log, theta_log, B, C):
    """Host-side prep: per-core in_maps. All heavy math in fp64 for accuracy."""
    u = np.asarray(u)
    lengths = np.asarray(lengths)
    nu = np.exp(np.asarray(nu_log, np.float64))
    theta = np.exp(np.asarray(theta_log, np.float64))
    r = np.exp(-nu)                                    # |lam|, (N,)

    j = np.arange(L, dtype=np.float64)
    ang = np.mod(theta[:, None] * j[None, :], 2 * np.pi)   # (N, L)
    cos_base = np.cos(ang).astype(np.float16)
    sin_base = np.sin(ang).astype(np.float16)

    Bcat = np.empty((H, 2 * N), np.float16)
    Bcat[:, :N] = np.asarray(B)[..., 0]
    Bcat[:, N:] = np.asarray(B)[..., 1]
    Ccat = np.empty((2 * N, H), np.float16)
    Ccat[:N] = np.asarray(C)[0]
    Ccat[N:] = -np.asarray(C)[1]
    rdec = r.reshape(CCH, P).T.astype(np.float32).copy()   # (128, 4)

    half = N // 2
    in_maps = []
    for b in range(BSZ):
        ln = int(lengths[b])
        ub = np.array(u[b], np.float32)
        if ln < L:
            ub[ln:, :] = 0.0
        uTh = np.ascontiguousarray(ub.T.astype(np.float16))
        cosb = cos_base.copy()
        sinb = sin_base.copy()
        if ln < L:
            cosb[:half, ln:] = 0
            sinb[:half, ln:] = 0
            cosb[half:, :L - ln] = 0
            sinb[half:, :L - ln] = 0
        in_maps.append({
            "uT": uTh, "cosT": cosb, "sinT": sinb,
            "rdec": rdec, "Bcat": Bcat, "Ccat": Ccat,
        })
    return in_maps


def kernel(u, lengths, nu_log, theta_log, B, C):
    global _CACHED
    from concourse.bass_utils import run_bass_kernel_spmd
    in_maps = prepare_inputs(u, lengths, nu_log, theta_log, B, C)
    if _CACHED is None:
        _CACHED = build_nc()
    res = run_bass_kernel_spmd(_CACHED, in_maps, list(range(BSZ)))
    y = np.stack([res.results[i]["y"] for i in range(BSZ)], axis=0)
    return y.astype(np.float32)


# revision 27
# speedup vs baseline: 1.0575x; 1.0100x over previous
"""Bidirectional complex-diagonal LRU (Linear Recurrent Unit) on 8 Trainium2 cores.

Math: lam = exp(-exp(nu_log) + i*exp(theta_log)) per channel n (N=512).
  Bu = einsum('blh,hn->bnl', u, B0 + iB1), masked to length.
  Forward scan over channels [0,256), backward (time-reversed) over [256,512).
  y = x.real @ C0 - x.imag @ C1, zeroed past each sequence length.

Device strategy (data-parallel, one batch per core):
  - Rotation trick: x_t = e^{i*th*t} * w_t turns the complex recurrence
    x_t = lam x_{t-1} + Bu_t into TWO real recurrences w_t = r w_{t-1} + v_t
    (r = |lam|), each a native DVE tensor_tensor_scan along the free dim.
  - Twiddle tables cos/sin(th*j) built on host in fp64 (exact phases), fp16 on
    device. Per-core masking (zero columns past the sequence length) is folded
    into the tables, so masking costs nothing on device.
  - Backward channels run on the reversed time axis; reversal happens inside
    the PSUM-evacuation copy (negative-stride AP) and the untwiddle writes.
  - All matmuls fp16 (PE rate equals bf16), accumulation in fp32 PSUM.
    x overwrites the spent v slabs (fwd) / cos+sin table slabs (bwd), so
    SBUF holds everything with no extra big buffers.
  - u streams in two pieces (outer quarters first) so both scan directions
    start immediately; untwiddle ADD-combines ride SWDGE accumulate DMAs.

Self-contained: hardcodes B=8, L=4096, H=N=512, 8 cores.
"""

import numpy as np
from contextlib import ExitStack

import concourse.bass as bass
import concourse.bacc as bacc
import concourse.mybir as mybir
import concourse.tile as tile

P = 128
L = 4096
H = 512
N = 512
BSZ = 8
SEG = 512                # Bu matmul / evac granularity (one PSUM bank)
NSEG = L // SEG          # 8
SLAB = 1024              # scan + untwiddle granularity
NSLAB = L // SLAB        # 4
TSLAB = 2048             # u-tile width (u streams in 2 pieces)
NTSLAB = L // TSLAB      # 2
KH = H // P              # 4 contraction chunks for Bu
NCH = 2 * N // P         # 8 real-channel chunks (re 0..3, im 4..7)
CCH = N // P             # 4 complex-channel chunks (0,1 fwd; 2,3 bwd)
NT = L // P              # 32 time blocks for the output matmul

F16 = mybir.dt.float16
F32 = mybir.dt.float32
MULT = mybir.AluOpType.mult
ADD = mybir.AluOpType.add

C_ORDER = [0, 2, 1, 3]

_CACHED = None


def _is_fwd_chunk(nch: int) -> bool:
    return (nch % 4) < 2


def build_nc():
    nc = bacc.Bacc("TRN2", target_bir_lowering=False, debug=False)
    uT = nc.declare_dram_parameter("uT", [H, L], F16, isOutput=False)
    cosT = nc.declare_dram_parameter("cosT", [N, L], F16, isOutput=False)
    sinT = nc.declare_dram_parameter("sinT", [N, L], F16, isOutput=False)
    rdec = nc.declare_dram_parameter("rdec", [P, CCH], F32, isOutput=False)
    Bcat = nc.declare_dram_parameter("Bcat", [H, 2 * N], F16, isOutput=False)
    Ccat = nc.declare_dram_parameter("Ccat", [2 * N, H], F16, isOutput=False)
    y = nc.declare_dram_parameter("y", [L, H], F32, isOutput=True)

    with tile.TileContext(nc) as tc, ExitStack() as ctx:
        const = ctx.enter_context(tc.tile_pool(name="const", bufs=1))
        big = ctx.enter_context(tc.tile_pool(name="big", bufs=1))
        upool = ctx.enter_context(tc.tile_pool(name="upool", bufs=1))
        pscr = ctx.enter_context(tc.tile_pool(name="pscr", bufs=4))
        qscr = ctx.enter_context(tc.tile_pool(name="qscr", bufs=6))
        wpool = ctx.enter_context(tc.tile_pool(name="wpool", bufs=10))
        ysb = ctx.enter_context(tc.tile_pool(name="ysb", bufs=3))
        bup = ctx.enter_context(tc.tile_pool(name="bup", bufs=6, space="PSUM"))
        yp = ctx.enter_context(tc.tile_pool(name="yp", bufs=2, space="PSUM"))

        # uT streamed in halves: cols [0:2048] then [2048:4096]
        u_t = [upool.tile([P, TSLAB], F16, tag=f"uT{k}", name=f"uT{k}")
               for k in range(KH)]
        cosb = [big.tile([P, L], F16, tag=f"cos{c}", name=f"cos{c}") for c in range(CCH)]
        sinb = [big.tile([P, L], F16, tag=f"sin{c}", name=f"sin{c}") for c in range(CCH)]
        v = [big.tile([P, L], F16, tag=f"v{j}", name=f"v{j}") for j in range(NCH)]
        bmat = [const.tile([P, 2 * N], F16, tag=f"B{k}", name=f"Bm{k}") for k in range(KH)]
        cmat = [const.tile([P, H], F16, tag=f"C{k}", name=f"Cm{k}") for k in range(NCH)]
        rdec_t = const.tile([P, CCH], F32, tag="rdec", name="rdec_t")

        # ---- constant DMAs (piece-A of u + Bcat first so Bu starts ASAP;
        #      table slab-0 pieces next for twiddle-in; Ccat last)
        nc.sync.dma_start(rdec_t[:], rdec[:])
        for k in range(KH):
            nc.sync.dma_start(bmat[k][:], Bcat[k * P:(k + 1) * P, :])
        # u piece A: t-cols [0:1024] + [3072:4096] -> tile cols [0:1024|1024:2048]
        for k in range(KH):
            nc.sync.dma_start(u_t[k][:, 0:SLAB], uT[k * P:(k + 1) * P, 0:SLAB])
            nc.sync.dma_start(u_t[k][:, SLAB:2 * SLAB],
                              uT[k * P:(k + 1) * P, 3 * SLAB:4 * SLAB])
        for c in range(CCH):
            nc.sync.dma_start(cosb[c][:, 0:SLAB], cosT[c * P:(c + 1) * P, 0:SLAB])
            nc.sync.dma_start(sinb[c][:, 0:SLAB], sinT[c * P:(c + 1) * P, 0:SLAB])
        for c in range(CCH):
            nc.sync.dma_start(cosb[c][:, SLAB:], cosT[c * P:(c + 1) * P, SLAB:])
            nc.sync.dma_start(sinb[c][:, SLAB:], sinT[c * P:(c + 1) * P, SLAB:])
        for k in range(NCH):
            nc.sync.dma_start(cmat[k][:], Ccat[k * P:(k + 1) * P, :])

        # ---- Phase A: Bu matmuls, evacuate into v slots (scan-time order) ----
        # u arrives in two pieces; piece A covers both directions' scan-slab 0.
        #   piece A: tsegs {0,1,7,6} at tile cols {0,512,1536,1024}
        #   piece B: tsegs {2,3,5,4} at tile cols {0,512,1536,1024}
        G_ORDER = [0, 4, 2, 6, 1, 5, 3, 7]
        UCOL = {0: 0, 1: 512, 7: 1536, 6: 1024, 2: 0, 3: 512, 5: 1536, 4: 1024}

        def do_group(nch, piece):
            tsegs = [0, 1, 7, 6] if piece == 0 else [2, 3, 5, 4]
            for ts in tsegs:
                ps = bup.tile([P, SEG], F32, name=f"bups{ts}", tag="bup")
                ucol = UCOL[ts]
                for k in range(KH):
                    nc.tensor.matmul(
                        ps[:],
                        bmat[k][:, nch * P:(nch + 1) * P],
                        u_t[k][:, ucol:ucol + SEG],
                        start=(k == 0), stop=(k == KH - 1),
                    )
                if _is_fwd_chunk(nch):
                    nc.scalar.copy(v[nch][:, ts * SEG:(ts + 1) * SEG], ps[:])
                else:
                    ss = NSEG - 1 - ts
                    dst = v[nch][:, ss * SEG:(ss + 1) * SEG]
                    nc.scalar.copy(dst[:, ::-1], ps[:])

        for nch in G_ORDER:
            do_group(nch, 0)
        # u piece B: t-cols [1024:2048] + [2048:3072]
        for k in range(KH):
            nc.sync.dma_start(u_t[k][:, 0:SLAB],
                              uT[k * P:(k + 1) * P, SLAB:2 * SLAB])
            nc.sync.dma_start(u_t[k][:, SLAB:2 * SLAB],
                              uT[k * P:(k + 1) * P, 2 * SLAB:3 * SLAB])
        for nch in G_ORDER:
            do_group(nch, 1)

        # ---- Phases B/C/D: twiddle-in + scan + untwiddle per (chunk, slab) ----
        prev_w = [None] * (2 * CCH)

        def twiddle_in(c, sb):
            jre, jim = c, c + CCH
            sl = slice(sb * SLAB, (sb + 1) * SLAB)
            cs, sn = cosb[c][:, sl], sinb[c][:, sl]
            vre, vim = v[jre][:, sl], v[jim][:, sl]
            p2 = pscr.tile([P, SLAB], F16, tag="p", name="p2")
            p3 = pscr.tile([P, SLAB], F16, tag="p", name="p3")
            p4 = pscr.tile([P, SLAB], F16, tag="p", name="p4")
            p1 = pscr.tile([P, SLAB], F16, tag="p", name="p1")
            nc.vector.tensor_mul(p4[:], sn, vre)
            nc.vector.tensor_mul(p1[:], cs, vre)
            nc.vector.tensor_mul(p2[:], sn, vim)
            nc.vector.tensor_mul(p3[:], cs, vim)
            nc.vector.tensor_add(vre, p1[:], p2[:])
            nc.vector.tensor_sub(vim, p3[:], p4[:])

        def scan_untw(c, sb):
            jre, jim = c, c + CCH
            sl = slice(sb * SLAB, (sb + 1) * SLAB)
            cs, sn = cosb[c][:, sl], sinb[c][:, sl]
            vre, vim = v[jre][:, sl], v[jim][:, sl]
            r_ap = rdec_t[:, c:c + 1].broadcast_to((P, SLAB))
            init_r = 0.0 if sb == 0 else prev_w[jre][:, SLAB - 1:SLAB]
            init_i = 0.0 if sb == 0 else prev_w[jim][:, SLAB - 1:SLAB]
            wr = wpool.tile([P, SLAB], F16, tag="w", name="wr")
            wi = wpool.tile([P, SLAB], F16, tag="w", name="wi")
            nc.vector.tensor_tensor_scan(wr[:], r_ap, vre, init_r,
                                         op0=MULT, op1=ADD)
            nc.vector.tensor_tensor_scan(wi[:], r_ap, vim, init_i,
                                         op0=MULT, op1=ADD)
            prev_w[jre], prev_w[jim] = wr, wi

            q1 = qscr.tile([P, SLAB], F16, tag="q", name="q1")
            q2 = qscr.tile([P, SLAB], F16, tag="q", name="q2")
            q4 = qscr.tile([P, SLAB], F16, tag="q", name="q4")
            if c < 2:
                # xr = q1 - q2 (DVE); xi written as q3 into the spent v slab
                # then += q4 via SWDGE accum dma
                nc.vector.tensor_mul(q1[:], cs, wr[:])
                nc.vector.tensor_mul(q2[:], sn, wi[:])
                nc.vector.tensor_mul(vim, sn, wr[:])
                nc.vector.tensor_mul(q4[:], cs, wi[:])
                nc.vector.tensor_sub(vre, q1[:], q2[:])
                nc.gpsimd.dma_start(vim, q4[:], accum_op=mybir.AluOpType.add)
            else:
                # reversed READS flip scan order back to t order for free;
                # t-slab (NSLAB-1-sb) lands at table slab sb, t-ascending.
                nc.vector.tensor_mul(q1[:], cs[:, ::-1], wr[:, ::-1])
                nc.vector.tensor_mul(q2[:], sn[:, ::-1], wi[:, ::-1])
                nc.vector.tensor_mul(q4[:], sn[:, ::-1], wr[:, ::-1])
                nc.vector.tensor_mul(sn, cs[:, ::-1], wi[:, ::-1])
                nc.vector.tensor_sub(cs, q1[:], q2[:])
                nc.gpsimd.dma_start(sn, q4[:], accum_op=mybir.AluOpType.add)

        # x source for the output matmul: real chunk k, time block i.
        # bwd x: t-slab (i//8) lives at table slab (NSLAB-1-i//8), t-ordered.
        def x_src(k: int, i: int):
            if _is_fwd_chunk(k):
                return v[k][:, i * P:(i + 1) * P]
            c = k % 4
            col = (NSLAB - 1 - i // 8) * SLAB + (i % 8) * P
            src = cosb[c] if k < 4 else sinb[c]
            return src[:, col:col + P]

        def y_slab(sj):
            for o in range(SLAB // P):
                i = sj * (SLAB // P) + o
                py = yp.tile([P, H], F32, name="py", tag="yp")
                for k in range(NCH):
                    nc.tensor.matmul(
                        py[:], x_src(k, i), cmat[k][:],
                        start=(k == 0), stop=(k == NCH - 1),
                    )
                yt = ysb.tile([P, H], F32, tag="y", name="yt")
                nc.scalar.copy(yt[:], py[:])
                nc.sync.dma_start(y[i * P:(i + 1) * P, :], yt[:])

        # split emission: fwd-chunk contributions first (their x is ready
        # early), bwd completes after the last backward scans; the 8 PSUM
        # banks (6 from the idle Bu pool + 2 from yp) hold in between.
        def y_slab_fwd(sj):
            tiles = []
            for o in range(SLAB // P):
                i = sj * (SLAB // P) + o
                pool = bup if o < 6 else yp
                tag = "bup" if o < 6 else "yp"
                py = pool.tile([P, H], F32, name="pys", tag=tag)
                first = True
                for k in (0, 1, 4, 5):
                    nc.tensor.matmul(py[:], x_src(k, i), cmat[k][:],
                                     start=first, stop=False)
                    first = False
                tiles.append(py)
            return tiles

        def y_slab_bwd(sj, tiles):
            for o in range(SLAB // P):
                i = sj * (SLAB // P) + o
                py = tiles[o]
                for k in (2, 3, 6, 7):
                    nc.tensor.matmul(py[:], x_src(k, i), cmat[k][:],
                                     start=False, stop=(k == 7))
                yt = ysb.tile([P, H], F32, tag="y", name="yt")
                nc.scalar.copy(yt[:], py[:])
                nc.sync.dma_start(y[i * P:(i + 1) * P, :], yt[:])

        # stream: slabs 0-1 per-direction blocked (2 live chains, not 4)
        twiddle_in(0, 0)
        twiddle_in(1, 0)
        scan_untw(0, 0)
        scan_untw(1, 0)
        twiddle_in(2, 0)
        twiddle_in(3, 0)
        scan_untw(2, 0)
        scan_untw(3, 0)
        twiddle_in(0, 1)
        twiddle_in(1, 1)
        scan_untw(0, 1)
        scan_untw(1, 1)
        twiddle_in(2, 1)
        twiddle_in(3, 1)
        scan_untw(2, 1)
        scan_untw(3, 1)
        # run the forward chains to completion first: Y2 and Y3 unblock as
        # soon as fwd slabs 2,3 land (their bwd deps finished in slabs 0,1);
        # then trail the backward chains for Y1, Y0.
        twiddle_in(0, 2)
        twiddle_in(1, 2)
        scan_untw(0, 2)
        scan_untw(1, 2)
        y_slab(2)            # fwd sb2 + bwd table-slab 1
        twiddle_in(0, 3)
        twiddle_in(1, 3)
        scan_untw(0, 3)
        scan_untw(1, 3)
        y_slab(3)            # fwd sb3 + bwd table-slab 0
        t1 = y_slab_fwd(1)
        twiddle_in(2, 2)
        twiddle_in(3, 2)
        scan_untw(2, 2)
        scan_untw(3, 2)
        y_slab_bwd(1, t1)    # + bwd table-slab 2
        t0_ = y_slab_fwd(0)
        twiddle_in(2, 3)
        twiddle_in(3, 3)
        scan_untw(2, 3)
        scan_untw(3, 3)
        y_slab_bwd(0, t0_)   # + bwd table-slab 3

    nc.compile()
    return nc


def prepare_inputs(u, lengths, nu_log, theta_log, B, C):
    """Host-side prep: per-core in_maps. All heavy math in fp64 for accuracy."""
    u = np.asarray(u)
    lengths = np.asarray(lengths)
    nu = np.exp(np.asarray(nu_log, np.float64))
    theta = np.exp(np.asarray(theta_log, np.float64))
    r = np.exp(-nu)                                    # |lam|, (N,)

    j = np.arange(L, dtype=np.float64)
    ang = np.mod(theta[:, None] * j[None, :], 2 * np.pi)   # (N, L)
    cos_base = np.cos(ang).astype(np.float16)
    sin_base = np.sin(ang).astype(np.float16)

    Bcat = np.empty((H, 2 * N), np.float16)
    Bcat[:, :N] = np.asarray(B)[..., 0]
    Bcat[:, N:] = np.asarray(B)[..., 1]
    Ccat = np.empty((2 * N, H), np.float16)
    Ccat[:N] = np.asarray(C)[0]
    Ccat[N:] = -np.asarray(C)[1]
    rdec = r.reshape(CCH, P).T.astype(np.float32).copy()   # (128, 4)

    half = N // 2
    in_maps = []
    for b in range(BSZ):
        ln = int(lengths[b])
        ub = np.array(u[b], np.float32)
        if ln < L:
            ub[ln:, :] = 0.0
        uTh = np.ascontiguousarray(ub.T.astype(np.float16))
        cosb = cos_base.copy()
        sinb = sin_base.copy()
        if ln < L:
            cosb[:half, ln:] = 0
            sinb[:half, ln:] = 0
            cosb[half:, :L - ln] = 0
            sinb[half:, :L - ln] = 0
        in_maps.append({
            "uT": uTh, "cosT": cosb, "sinT": sinb,
            "rdec": rdec, "Bcat": Bcat, "Ccat": Ccat,
        })
    return in_maps


def kernel(u, lengths, nu_log, theta_log, B, C):
    global _CACHED
    from concourse.bass_utils import run_bass_kernel_spmd
    in_maps = prepare_inputs(u, lengths, nu_log, theta_log, B, C)
    if _CACHED is None:
        _CACHED = build_nc()
    res = run_bass_kernel_spmd(_CACHED, in_maps, list(range(BSZ)))
    y = np.stack([res.results[i]["y"] for i in range(BSZ)], axis=0)
    return y.astype(np.float32)
